# revision 15
# baseline (speedup 1.0000x reference)
"""Trainium2 Bass kernel for nn_AttentionApproximator (sparse_attention).

Math (per batch b):
  scores = relu(full @ sw1 + sb1) @ sw2 + sb2            [S]
  top_idx = top_k(scores, k=204)                          (set only matters)
  sel     = full[top_idx]                                 [k, d]
  q_part  = full @ mw1[:d]                                [S, 64]
  kvb     = sel @ (mw1[d:2d] + mw1[2d:]) + mb1            [k, 64]
  h1      = relu(q_part[s] + kvb[j])                      [S, k, 64]
  h2      = relu(h1 @ mw2 + mb2)                          [S, k, 32]
  out     = mean_j(h2) @ mw3 + mb3                        [S, d]
       (the mw3 matmul commutes with the j-mean)

Device strategy (8 cores, SPMD): core c handles batch b=c//2, query rows
h=c%2 (1024 of 2048).  Top-k selection is computed on-device per core via
exact ranks: rank_i = #{j: s_j > s_i}; the top-204 set is {rank < 204} and
rank_i is also the compaction slot, so the gather is a one-hot matmul.
All layout changes are done via host-side input prep (transposed copies)
or matmul tricks -- no on-device transposes.
"""

import os
from contextlib import ExitStack

import numpy as np

B, S, D = 4, 2048, 16
DA = D + 1               # augmented with ones row
K = 204                  # top-k  (int(2048*0.1))
KP = K // 2              # 102 slot-pairs
H1 = 64
H2 = 32
SH = S // 2              # 1024 query rows per core
NCH = S // 128           # 16 token chunks
N_CORES = 8
INV_K = float(np.float32(1.0) / np.float32(K))

_cache = {}


def _build_module():
    import concourse.mybir as mybir
    import concourse.tile as tile
    from concourse import bacc

    fp32 = mybir.dt.float32
    Alu = mybir.AluOpType
    Act = mybir.ActivationFunctionType

    nc = bacc.Bacc("TRN2", target_bir_lowering=False, debug=False,
                   num_devices=N_CORES)

    # ---- DRAM I/O ----
    d_fbT = nc.dram_tensor("fbT", [DA, S], fp32, kind="ExternalInput").ap()
    d_fqT = nc.dram_tensor("fqT", [D, SH], fp32, kind="ExternalInput").ap()
    d_wq = nc.dram_tensor("wq", [D, H1], fp32, kind="ExternalInput").ap()
    d_wkv = nc.dram_tensor("wkv", [DA, H1], fp32, kind="ExternalInput").ap()
    d_sw1a = nc.dram_tensor("sw1a", [DA, H2], fp32, kind="ExternalInput").ap()
    d_sw2c = nc.dram_tensor("sw2c", [H2, 1], fp32, kind="ExternalInput").ap()
    d_ones = nc.dram_tensor("ones128", [1, 128], fp32, kind="ExternalInput").ap()
    d_bdmw2 = nc.dram_tensor("bdmw2", [128, H1], fp32, kind="ExternalInput").ap()
    d_mb24 = nc.dram_tensor("mb24", [128, 1], fp32, kind="ExternalInput").ap()
    d_mw34 = nc.dram_tensor("mw34", [128, D], fp32, kind="ExternalInput").ap()
    d_mb3f = nc.dram_tensor("mb3f", [128, 1], fp32, kind="ExternalInput").ap()
    d_iotaE = nc.dram_tensor("iotaE", [128, KP], fp32, kind="ExternalInput").ap()
    d_iotaO = nc.dram_tensor("iotaO", [128, KP], fp32, kind="ExternalInput").ap()

    d_outT = nc.dram_tensor("outT", [D, SH], fp32, kind="ExternalOutput").ap()

    dbg = bool(int(os.environ.get("KERNEL_DEBUG", "0")))
    if dbg:
        d_rank = nc.dram_tensor("dbg_rank", [128, NCH], fp32,
                                kind="ExternalOutput").ap()
        d_kvb2 = nc.dram_tensor("dbg_kvb2", [128, KP], fp32,
                                kind="ExternalOutput").ap()
        d_stok = nc.dram_tensor("dbg_stok", [128, NCH], fp32,
                                kind="ExternalOutput").ap()

    with tile.TileContext(nc) as tc:
        with (
            ExitStack() as es,
            tc.tile_pool(name="const", bufs=1) as cpool,
            tc.tile_pool(name="sel", bufs=1) as spool,
            tc.tile_pool(name="scratch", bufs=2) as scpool,
            tc.tile_pool(name="h1p", bufs=3) as h1pool,
            tc.tile_pool(name="h2p", bufs=2) as h2pool,
        ):
            # ---- load constants / inputs to SBUF ----
            sb_fbT = cpool.tile([DA, S], fp32)
            nc.sync.dma_start(sb_fbT[:], d_fbT[:])
            sb_fqT = cpool.tile([D, SH], fp32)
            nc.sync.dma_start(sb_fqT[:], d_fqT[:])
            sb_wq = cpool.tile([D, H1], fp32)
            nc.sync.dma_start(sb_wq[:], d_wq[:])
            sb_wkv = cpool.tile([DA, H1], fp32)
            nc.sync.dma_start(sb_wkv[:], d_wkv[:])
            sb_sw1a = cpool.tile([DA, H2], fp32)
            nc.sync.dma_start(sb_sw1a[:], d_sw1a[:])
            sb_sw2c = cpool.tile([H2, 1], fp32)
            nc.sync.dma_start(sb_sw2c[:], d_sw2c[:])
            sb_ones = cpool.tile([1, 128], fp32)
            nc.sync.dma_start(sb_ones[:], d_ones[:])
            sb_bdmw2 = cpool.tile([128, H1], fp32)
            nc.sync.dma_start(sb_bdmw2[:], d_bdmw2[:])
            sb_mb24 = cpool.tile([128, 1], fp32)
            nc.sync.dma_start(sb_mb24[:], d_mb24[:])
            sb_mw34 = cpool.tile([128, D], fp32)
            nc.sync.dma_start(sb_mw34[:], d_mw34[:])
            sb_mb3f = cpool.tile([128, 1], fp32)
            nc.sync.dma_start(sb_mb3f[:], d_mb3f[:])
            sb_iotaE = cpool.tile([128, KP], fp32)
            nc.sync.dma_start(sb_iotaE[:], d_iotaE[:])
            sb_iotaO = cpool.tile([128, KP], fp32)
            nc.sync.dma_start(sb_iotaO[:], d_iotaO[:])

            # ---- stage A: scores hidden layer + per-token scores ----
            with tc.tile_pool(name="psA", bufs=1, space="PSUM") as pA:
                ps_H = pA.tile([H2, S], fp32)          # 4 banks
                for n in range(4):
                    sl = slice(n * 512, (n + 1) * 512)
                    nc.tensor.matmul(ps_H[:, sl], sb_sw1a[:], sb_fbT[:, sl],
                                     start=True, stop=True)
                sb_H = spool.tile([H2, S], fp32)
                for n in range(4):
                    sl = slice(n * 512, (n + 1) * 512)
                    nc.scalar.activation(sb_H[:, sl], ps_H[:, sl], Act.Relu)

            # scores row [1, S] -- the single source of truth for scores.
            # Both layouts below are derived from it by exact data movement
            # (x1.0 matmul / transpose), so comparisons are self-consistent
            # on hardware despite matmul rounding.
            with tc.tile_pool(name="psA15", bufs=1, space="PSUM") as pA15:
                ps_sr = pA15.tile([1, S], fp32)        # 4 banks
                for n in range(4):
                    sl = slice(n * 512, (n + 1) * 512)
                    nc.tensor.matmul(ps_sr[:, sl], sb_sw2c[:], sb_H[:, sl],
                                     start=True, stop=True)
                sb_sr = spool.tile([1, S], fp32)
                nc.scalar.copy(sb_sr[:], ps_sr[:])

            with tc.tile_pool(name="psA2", bufs=1, space="PSUM") as pA2:
                # scores, token-major [128, 16] via PE transposes
                ps_stok = pA2.tile([128, NCH], fp32)   # 1 bank
                for c in range(NCH):
                    nc.tensor.transpose(ps_stok[:, c:c + 1],
                                        sb_sr[0:1, c * 128:(c + 1) * 128],
                                        sb_ones[0:1, 0:1])
                sb_stok = spool.tile([128, NCH], fp32)
                nc.scalar.copy(sb_stok[:], ps_stok[:])

                # scores broadcast [128, S]: ones-column x scores row (exact)
                ps_bc = pA2.tile([128, S], fp32)       # 4 banks
                for n in range(4):
                    sl = slice(n * 512, (n + 1) * 512)
                    nc.tensor.matmul(ps_bc[:, sl], sb_ones[:],
                                     sb_sr[0:1, sl], start=True, stop=True)
                sb_bc = spool.tile([128, S], fp32)
                nc.vector.tensor_copy(sb_bc[:], ps_bc[:])

            # ---- stage B: exact ranks (token-major) ----
            sb_rank = spool.tile([128, NCH], fp32)
            for c in range(NCH):
                scr = scpool.tile([128, S], fp32, tag="cmp_scr")
                nc.vector.tensor_scalar(
                    scr[:], sb_bc[:], sb_stok[:, c:c + 1], 0.0,
                    Alu.is_gt, Alu.add, accum_out=sb_rank[:, c:c + 1])

            # ---- stage C: kv for all tokens + one-hot gather of top-K ----
            with tc.tile_pool(name="psC", bufs=2, space="PSUM") as pC, \
                 tc.tile_pool(name="psCsel", bufs=1, space="PSUM") as pCs:
                sb_kvtok = spool.tile([128, NCH * H1], fp32)
                for c in range(NCH):
                    ps_kv = pC.tile([128, H1], fp32, tag="kvtok")
                    nc.tensor.matmul(ps_kv[:],
                                     sb_fbT[:, c * 128:(c + 1) * 128],
                                     sb_wkv[:], start=True, stop=True)
                    nc.vector.tensor_copy(sb_kvtok[:, c * H1:(c + 1) * H1],
                                          ps_kv[:])

                # full-bank pitch (512 f32) so base-partition-64 slices stay
                # bank-aligned in the has_written/pending-zero bookkeeping
                ps_kvsel_full = pCs.tile([128, 512], fp32)   # 1 bank
                ps_kvsel = ps_kvsel_full[:, 0:KP]
                for c in range(NCH):
                    ohE = scpool.tile([128, KP], fp32, tag="ohE")
                    nc.vector.tensor_scalar(ohE[:], sb_iotaE[:],
                                            sb_rank[:, c:c + 1], None,
                                            Alu.is_equal)
                    ohO = scpool.tile([128, KP], fp32, tag="ohO")
                    nc.vector.tensor_scalar(ohO[:], sb_iotaO[:],
                                            sb_rank[:, c:c + 1], None,
                                            Alu.is_equal)
                    kvch = sb_kvtok[:, c * H1:(c + 1) * H1]
                    nc.tensor.matmul(ps_kvsel[0:H1, :], kvch, ohE[:],
                                     start=(c == 0), stop=False,
                                     skip_group_check=True)
                    nc.tensor.matmul(ps_kvsel[H1:128, :], kvch, ohO[:],
                                     start=(c == 0), stop=(c == NCH - 1),
                                     skip_group_check=True)
                sb_kvb2 = spool.tile([128, KP], fp32)
                nc.scalar.copy(sb_kvb2[:], ps_kvsel[:])

            pmain = es.enter_context(
                tc.tile_pool(name="main_psum", bufs=2, space="PSUM"))
            pout = es.enter_context(
                tc.tile_pool(name="out_psum", bufs=1, space="PSUM"))

            # ---- stage D: query part, stacked twice [128, SH] ----
            ps_q = pmain.tile([128, SH], fp32, tag="hps")
            for half in range(2):
                rows = slice(half * H1, (half + 1) * H1)
                for n in range(2):
                    sl = slice(n * 512, (n + 1) * 512)
                    nc.tensor.matmul(ps_q[rows, sl], sb_wq[:], sb_fqT[:, sl],
                                     start=True, stop=True)
            sb_qT2 = cpool.tile([128, SH], fp32)
            nc.vector.tensor_copy(sb_qT2[:], ps_q[:])

            if dbg:
                nc.sync.dma_start(d_rank[:], sb_rank[:])
                nc.sync.dma_start(d_kvb2[:], sb_kvb2[:])
                nc.sync.dma_start(d_stok[:], sb_stok[:])

            # ---- stage E: main pairwise loop ----
            # 102 pair-iters p (tokens 2p, 2p+1); two iters packed per psum
            ps_out = pout.tile([128, SH], fp32, tag="outacc")  # 2 banks
            n_w1_act = 6   # a few W1 instances on ACT to balance engines
            for g in range(KP // 2):                  # 51 groups
                ps_h = pmain.tile([128, SH], fp32, tag="hps")
                for half in range(2):
                    p = 2 * g + half
                    h1t = h1pool.tile([128, SH], fp32, tag="h1")
                    if p % (KP // n_w1_act) == 0:
                        nc.scalar.activation(h1t[:], sb_qT2[:], Act.Relu,
                                             bias=sb_kvb2[:, p:p + 1])
                    else:
                        nc.vector.tensor_scalar(h1t[:], sb_qT2[:],
                                                sb_kvb2[:, p:p + 1], 0.0,
                                                Alu.add, Alu.max)
                    rows = slice(half * H1, (half + 1) * H1)
                    for n in range(2):
                        sl = slice(n * 512, (n + 1) * 512)
                        nc.tensor.matmul(ps_h[rows, sl], sb_bdmw2[:],
                                         h1t[:, sl], start=True, stop=True)
                h2t = h2pool.tile([128, SH], fp32, tag="h2")
                nc.scalar.activation(h2t[:], ps_h[:], Act.Relu,
                                     bias=sb_mb24[:])
                for n in range(2):
                    sl = slice(n * 512, (n + 1) * 512)
                    nc.tensor.matmul(ps_out[64:64 + D, sl], sb_mw34[:],
                                     h2t[:, sl], start=(g == 0),
                                     stop=(g == KP // 2 - 1),
                                     skip_group_check=True)

            # ---- stage F: scale + bias + store ----
            sb_out = spool.tile([128, SH], fp32)
            nc.scalar.activation(sb_out[64:64 + D, :], ps_out[64:64 + D, :],
                                 Act.Identity, bias=sb_mb3f[64:64 + D, :],
                                 scale=INV_K)
            nc.sync.dma_start(d_outT[:], sb_out[64:64 + D, :])

    nc.compile()
    return nc


def _host_inputs(full, sw1, sb1, sw2, sb2, mw1, mb1, mw2, mb2, mw3, mb3):
    """Build the 8 per-core input maps (host-side sharding + layout prep)."""
    f32 = np.float32
    full = np.asarray(full, dtype=f32)
    ones_row = np.ones((1, S), dtype=f32)
    wq = np.ascontiguousarray(np.asarray(mw1[:D], dtype=f32))
    wkv = np.concatenate([np.asarray(mw1[D:2 * D] + mw1[2 * D:], dtype=f32),
                          np.asarray(mb1, dtype=f32)[None, :]], axis=0)
    sw1a = np.concatenate([np.asarray(sw1, dtype=f32),
                           np.asarray(sb1, dtype=f32)[None, :]], axis=0)
    sw2c = np.ascontiguousarray(np.asarray(sw2, dtype=f32).reshape(H2, 1))
    ones128 = np.ones((1, 128), dtype=f32)
    bdmw2 = np.zeros((128, H1), dtype=f32)
    bdmw2[0:H1, 0:H2] = mw2
    bdmw2[H1:128, H2:H1] = mw2
    mb24 = np.tile(np.asarray(mb2, dtype=f32), 4)[:, None].copy()
    mw34 = np.ascontiguousarray(np.tile(np.asarray(mw3, dtype=f32), (4, 1)))
    mb3f = np.tile(np.asarray(mb3, dtype=f32), 8)[:, None].copy()
    iotaE = np.broadcast_to(np.arange(0, K, 2, dtype=f32), (128, KP)).copy()
    iotaO = np.broadcast_to(np.arange(1, K, 2, dtype=f32), (128, KP)).copy()

    shared = dict(wq=wq, wkv=wkv, sw1a=sw1a, sw2c=sw2c, ones128=ones128,
                  bdmw2=bdmw2, mb24=mb24, mw34=mw34, mb3f=mb3f,
                  iotaE=iotaE, iotaO=iotaO)
    in_maps = []
    for c in range(N_CORES):
        b, h = c // 2, c % 2
        fbT = np.concatenate(
            [np.ascontiguousarray(full[b].T), ones_row], axis=0)
        fqT = np.ascontiguousarray(full[b, h * SH:(h + 1) * SH, :].T)
        m = dict(shared)
        m["fbT"] = fbT
        m["fqT"] = fqT
        in_maps.append(m)
    return in_maps


def get_module():
    if "nc" not in _cache:
        _cache["nc"] = _build_module()
    return _cache["nc"]


def run_cores(in_maps):
    from concourse.bass_utils import run_bass_kernel_spmd
    nc = get_module()
    return run_bass_kernel_spmd(nc, in_maps, list(range(N_CORES))).results


def kernel(full, sw1, sb1, sw2, sb2, mw1, mb1, mw2, mb2, mw3, mb3):
    in_maps = _host_inputs(full, sw1, sb1, sw2, sb2, mw1, mb1, mw2, mb2,
                           mw3, mb3)
    results = run_cores(in_maps)
    out = np.empty((B, S, D), dtype=np.float32)
    for c in range(N_CORES):
        b, h = c // 2, c % 2
        out[b, h * SH:(h + 1) * SH, :] = results[c]["outT"].T
    return out
